# revision 9
# baseline (speedup 1.0000x reference)
"""Trainium2 Bass kernel for nn_CMix_x060moe (RWKV CMix + hash-routed MoE).

Strategy: expert-sharded SPMD over 8 NeuronCores. Hash routing depends only
on token_ids, so the host computes the token->expert assignment as part of
sharding: core e receives exactly 2048 tokens (expert e's kept tokens in
FIFO order, padded with capacity-dropped tokens from anywhere, mask=0 for
those). Each core computes the dense squared-ReLU FFN, the sigmoid
receptance and its own expert's FFN for its 2048 tokens; the host scatters
rows back. No collectives needed and the load is perfectly balanced.

The token-shift (xk = x + (xprev-x)*maa) is folded into the host-side
gather: the kernel receives xk directly, in bf16, C-major ([C, tokens]).
In the reference init time_maa_k == time_maa_r, so xr == xk and the
receptance shares the same tiles (a separate-xr variant is kept for
safety). All matmuls run in bf16 (full 2-rows/cycle PE rate); PSUM
accumulation and the kv accumulator stay fp32.
"""

import os

import ml_dtypes
import numpy as np

import concourse.mybir as mybir
import concourse.tile as tile
from concourse import bacc
from concourse.bass_utils import run_bass_kernel_spmd

LAST_RESULTS = None  # set on every kernel() call; holds BassKernelResults

B, T, C = 8, 2048, 1024
DFF, DFFE = 4096, 2048
E = 8
HASH_PRIME = 5099
CAP = (B * T) // E  # 2048
N = B * T

P = 128               # partitions
SB = CAP              # one pass over all tokens
TB = 512              # matmul token width (psum bank)
H = SB // TB          # 4 column chunks
CT = C // P           # 8  C-tiles
MT_D = DFF // P       # 32 dense-hidden tiles
MT_E = DFFE // P      # 16 expert-hidden tiles
GD = 8                # dense second-layer contraction groups
GE = 4                # expert second-layer contraction groups
HD = MT_D // GD       # 4 k-tiles per dense group
HE = MT_E // GE       # 4 k-tiles per expert group

F32 = mybir.dt.float32
BF16 = mybir.dt.bfloat16

_COMPILED = {}


def _build(shared_xr: bool):
    nc = bacc.Bacc(trn_type="TRN2")

    xkd = nc.dram_tensor("xk", [CT, P, SB], BF16, kind="ExternalInput")
    if not shared_xr:
        xrd = nc.dram_tensor("xr", [CT, P, SB], BF16, kind="ExternalInput")
    maskd = nc.dram_tensor("maskd", [P, SB], BF16, kind="ExternalInput")
    # weights, host-tiled p-major: w*[m][p][k*P+q] = W[k*P+p, m*P+q]
    wk = nc.dram_tensor("wk", [MT_D, P, CT * P], BF16, kind="ExternalInput")
    wv = nc.dram_tensor("wv", [CT, P, MT_D * P], BF16, kind="ExternalInput")
    wr = nc.dram_tensor("wr", [CT, P, CT * P], BF16, kind="ExternalInput")
    wek = nc.dram_tensor("wek", [MT_E, P, CT * P], BF16, kind="ExternalInput")
    wev = nc.dram_tensor("wev", [CT, P, MT_E * P], BF16, kind="ExternalInput")
    yout = nc.dram_tensor("y", [CT, P, SB], F32, kind="ExternalOutput")

    with tile.TileContext(nc) as tc:
        with (
            tc.tile_pool(name="const", bufs=1) as constp,
            tc.tile_pool(name="acts", bufs=1) as acts,
            tc.tile_pool(name="wfirst", bufs=8) as wfp,
            tc.tile_pool(name="wsecond", bufs=10) as wsp,
            tc.tile_pool(name="tmp", bufs=3) as tmpp,
            tc.tile_pool(name="outp", bufs=3) as outp,
            tc.tile_pool(name="ps1", bufs=4, space="PSUM") as ps1,
            tc.tile_pool(name="ps2", bufs=4, space="PSUM") as ps2,
        ):
            tmask = constp.tile([P, SB], BF16)

            # ---- inputs (host-precomputed token-shift, bf16 C-major) ----
            # Quarter-major order so the first dense group's inputs land
            # first; each quarter chunk rides its own DMA engine.
            xk = [acts.tile([P, SB], BF16, tag=f"xk{i}", name=f"xk{i}")
                  for i in range(CT)]
            for q in range(H):
                cols = slice(q * TB, (q + 1) * TB)
                if q == 0:
                    # critical for the first matmuls: halve and issue the
                    # descriptors from two rings in parallel
                    lo = slice(0, TB // 2)
                    hi = slice(TB // 2, TB)
                    for ct in range(CT):
                        nc.sync.dma_start(xk[ct][:, lo], xkd[ct, :, lo])
                    for ct in range(CT):
                        nc.scalar.dma_start(xk[ct][:, hi], xkd[ct, :, hi])
                else:
                    for ct in range(CT):
                        nc.sync.dma_start(xk[ct][:, cols], xkd[ct, :, cols])
            if shared_xr:
                xr = xk
            else:
                xr = [acts.tile([P, SB], BF16, tag=f"xr{i}", name=f"xr{i}")
                      for i in range(CT)]
                for q in range(H):
                    cols = slice(q * TB, (q + 1) * TB)
                    for ct in range(CT):
                        nc.sync.dma_start(xr[ct][:, cols], xrd[ct, :, cols])

            kv = [acts.tile([P, SB], F32, tag=f"kv{i}", name=f"kv{i}")
                  for i in range(CT)]

            def ffn_group(g, par, nk, wfirst, wsecond, wsec_mt, masked,
                          weng=None, wsplit=1):
                """One contraction group: first layer -> squared relu
                (optionally masked) -> second layer accumulated into kv."""
                kt = [acts.tile([P, SB], BF16, tag=f"kt{i}_{par}",
                                name=f"kt{g}_{i}")
                      for i in range(nk)]
                weng = weng or nc.sync
                wts = []
                for i in range(nk):
                    wt = wfp.tile([P, CT * P], BF16, tag="wA", name=f"wA{g}_{i}")
                    wcols = (CT * P) // wsplit
                    for s in range(wsplit):
                        cs = slice(s * wcols, (s + 1) * wcols)
                        weng.dma_start(wt[:, cs], wfirst[g * nk + i][:, cs])
                    wts.append(wt)
                for h in range(H):
                    tok = slice(h * TB, (h + 1) * TB)
                    for i in range(nk):
                        pd = ps1.tile([P, TB], F32, tag="ps1", name="pd")
                        for k in range(CT):
                            nc.tensor.matmul(
                                pd[:], wts[i][:, k * P:(k + 1) * P],
                                xk[k][:, tok],
                                start=(k == 0), stop=(k == CT - 1),
                            )
                        rl = tmpp.tile([P, TB], BF16, tag="rl", name="rl")
                        nc.scalar.activation(
                            rl[:], pd[:], mybir.ActivationFunctionType.Relu
                        )
                        if masked:
                            rlm = tmpp.tile([P, TB], BF16, tag="rlm", name="rlm")
                            nc.vector.tensor_tensor(
                                out=rlm[:], in0=rl[:], in1=tmask[:, tok],
                                op=mybir.AluOpType.mult,
                            )
                            nc.vector.tensor_tensor(
                                out=kt[i][:, tok], in0=rl[:], in1=rlm[:],
                                op=mybir.AluOpType.mult,
                            )
                        else:
                            nc.vector.tensor_tensor(
                                out=kt[i][:, tok], in0=rl[:], in1=rl[:],
                                op=mybir.AluOpType.mult,
                            )
                # second layer, h-outer so deps resolve early; all wsec
                # group tiles stay resident (wsp bufs covers nk*... CT tiles)
                swts = []
                for m in range(CT):
                    wt = wsp.tile([P, nk * P], BF16, tag="wB", name=f"wB{g}_{m}")
                    nc.gpsimd.dma_start(
                        wt[:], wsecond[m, :, g * nk * P:(g + 1) * nk * P]
                    )
                    swts.append(wt)
                first = not masked and g == 0
                for h in range(H):
                    tok = slice(h * TB, (h + 1) * TB)
                    for m in range(CT):
                        pv = ps2.tile([P, TB], F32, tag="ps2", name="pv")
                        for k in range(nk):
                            nc.tensor.matmul(
                                pv[:], swts[m][:, k * P:(k + 1) * P],
                                kt[k][:, tok],
                                start=(k == 0), stop=(k == nk - 1),
                            )
                        if first:
                            nc.vector.tensor_copy(kv[m][:, tok], pv[:])
                        else:
                            nc.vector.tensor_tensor(
                                out=kv[m][:, tok], in0=pv[:],
                                in1=kv[m][:, tok],
                                op=mybir.AluOpType.add,
                            )

            # ---- dense: k = relu(xk@Wk)^2 ; kv = k @ Wv  (grouped) ----
            # Group 0's weights ride the gpsimd ring (in parallel with the
            # xk descriptors on sync), split so each piece gets its own DMA
            # engine: minimizes the startup stall before the first matmul.
            ffn_group(0, 0, HD, wk, wv, MT_D, masked=False,
                      weng=nc.gpsimd, wsplit=2)
            # Mask is first needed by the expert phase; issue off-critical.
            nc.scalar.dma_start(tmask[:], maskd[:])
            for g in range(1, GD):
                ffn_group(g, g % 2, HD, wk, wv, MT_D, masked=False)

            # ---- expert: kv += mask * (relu(xk@Wek)^2 @ Wev) ----
            for g in range(GE):
                ffn_group(g, (GD + g) % 2, HE, wek, wev, MT_E, masked=True)

            # ---- receptance last: y = sigmoid(xr @ Wr) * kv ----
            for m in range(CT):
                wt = wfp.tile([P, CT * P], BF16, tag="wA", name=f"wr{m}")
                nc.sync.dma_start(wt[:], wr[m])
                for h in range(H):
                    tok = slice(h * TB, (h + 1) * TB)
                    pr = ps1.tile([P, TB], F32, tag="ps1", name="pr")
                    for k in range(CT):
                        nc.tensor.matmul(
                            pr[:], wt[:, k * P:(k + 1) * P], xr[k][:, tok],
                            start=(k == 0), stop=(k == CT - 1),
                        )
                    rm = tmpp.tile([P, TB], F32, tag="rm", name="rm")
                    nc.scalar.activation(
                        rm[:], pr[:], mybir.ActivationFunctionType.Sigmoid
                    )
                    yo = outp.tile([P, TB], F32, tag="yo", name="yo")
                    nc.vector.tensor_tensor(
                        out=yo[:], in0=kv[m][:, tok], in1=rm[:],
                        op=mybir.AluOpType.mult,
                    )
                    # split output DMAs; the final chunks go out via four
                    # different rings so descriptor issue isn't serialized
                    # behind one engine at the kernel tail
                    if m == CT - 1:
                        engs = [nc.gpsimd, nc.sync, nc.scalar, nc.sync]
                        for s, eng in enumerate(engs):
                            cs = slice(s * TB // 4, (s + 1) * TB // 4)
                            eng.dma_start(yout[m, :, tok][:, cs], yo[:, cs])
                    else:
                        for s in range(2):
                            cs = slice(s * TB // 2, (s + 1) * TB // 2)
                            nc.gpsimd.dma_start(
                                yout[m, :, tok][:, cs], yo[:, cs]
                            )

    nc.compile()
    return nc


def _routing(token_ids: np.ndarray):
    """Token -> (per-core global token list [E, CAP], per-core keep mask)."""
    tid = token_ids.reshape(N).astype(np.int64)
    eidx = (tid * HASH_PRIME) % E
    order = np.argsort(eidx, kind="stable")  # FIFO within expert
    counts = np.bincount(eidx, minlength=E)
    starts = np.zeros(E + 1, np.int64)
    np.cumsum(counts, out=starts[1:])

    token_lists = np.empty((E, CAP), np.int64)
    masks = np.zeros((E, CAP), np.float32)
    dropped = []
    fill_needed = []
    for e in range(E):
        grp = order[starts[e]:starts[e + 1]]
        nk = min(len(grp), CAP)
        token_lists[e, :nk] = grp[:nk]
        masks[e, :nk] = 1.0
        dropped.append(grp[CAP:])
        fill_needed.append(CAP - nk)
    dropped = (
        np.concatenate(dropped) if dropped else np.empty(0, np.int64)
    )
    pos = 0
    for e in range(E):
        need = fill_needed[e]
        if need:
            token_lists[e, CAP - need:] = dropped[pos:pos + need]
            pos += need
    assert pos == len(dropped)
    return token_lists, masks


def _tile_first(W, mt):
    """[C, M] -> [mt, P, CT*P] with w[m][p][k*P+q] = W[k*P+p, m*P+q]."""
    ct = W.shape[0] // P
    return np.ascontiguousarray(
        W.reshape(ct, P, mt, P).transpose(2, 1, 0, 3).reshape(mt, P, ct * P)
        .astype(ml_dtypes.bfloat16)
    )


def _tile_second(W, ct_out):
    """[K, M] -> [ct_out, P, KT*P] with w[m][p][k*P+q] = W[k*P+p, m*P+q]."""
    kt = W.shape[0] // P
    return np.ascontiguousarray(
        W.reshape(kt, P, ct_out, P).transpose(2, 1, 0, 3)
        .reshape(ct_out, P, kt * P).astype(ml_dtypes.bfloat16)
    )


def kernel(x, shift_state, token_ids, time_maa_k, time_maa_r, Wk, Wv, Wr, Wek, Wev):
    x = np.asarray(x, np.float32)
    shift_state = np.asarray(shift_state, np.float32)
    time_maa_k = np.asarray(time_maa_k, np.float32)
    time_maa_r = np.asarray(time_maa_r, np.float32)
    shared_xr = bool(np.array_equal(time_maa_k, time_maa_r))

    if shared_xr not in _COMPILED:
        _COMPILED[shared_xr] = _build(shared_xr)
    nc = _COMPILED[shared_xr]

    token_lists, masks = _routing(np.asarray(token_ids))

    xf = x.reshape(N, C)
    xprev_f = np.empty_like(xf)
    xprev_f[1:] = xf[:-1]
    xprev_f[np.arange(B) * T] = shift_state

    dx = xprev_f - xf
    xk_full = xf + dx * time_maa_k[None, :]
    if not shared_xr:
        xr_full = xf + dx * time_maa_r[None, :]

    wk_t = _tile_first(np.asarray(Wk, np.float32), MT_D)
    wr_t = _tile_first(np.asarray(Wr, np.float32), CT)
    wv_t = _tile_second(np.asarray(Wv, np.float32), CT)
    Wek = np.asarray(Wek, np.float32)
    Wev = np.asarray(Wev, np.float32)

    def ctmajor(rows):  # [CAP, C] -> [CT, P, CAP] bf16
        return np.ascontiguousarray(
            rows.T.reshape(CT, P, CAP).astype(ml_dtypes.bfloat16)
        )

    in_maps = []
    for e in range(E):
        L = token_lists[e]
        m = dict(
            xk=ctmajor(xk_full[L]),
            maskd=np.ascontiguousarray(
                np.broadcast_to(masks[e], (P, CAP))
            ).astype(ml_dtypes.bfloat16),
            wk=wk_t,
            wv=wv_t,
            wr=wr_t,
            wek=_tile_first(Wek[e], MT_E),
            wev=_tile_second(Wev[e], CT),
        )
        if not shared_xr:
            m["xr"] = ctmajor(xr_full[L])
        in_maps.append(m)

    res = run_bass_kernel_spmd(
        nc, in_maps, core_ids=list(range(E)),
        trace=bool(os.environ.get("KERNEL_TRACE")),
    )
    global LAST_RESULTS
    LAST_RESULTS = res

    y = np.empty((N, C), np.float32)
    for e in range(E):
        y[token_lists[e]] = res.results[e]["y"].reshape(C, CAP).T
    return y.reshape(B, T, C)
